# revision 1
# baseline (speedup 1.0000x reference)
"""Trainium2 Bass kernel for KernelizedSupCon loss (B=2048, V=2, D=512, 8 cores).

Strategy (data-parallel over anchor rows, per sharding hint):
  - N = B*V = 4096 anchor rows; core c owns rows [c*512, (c+1)*512).
  - Host precomputes, per core: transposed/rotated features fT [D=512, N=4096]
    (so both matmul operands come from one SBUF-resident tensor), the
    delta-count matrix and positive-mask row-blocks (rotated so the diagonal
    block is always column-tile 0 -> one uniform SPMD program), and the
    positive-mask row sums P.
  - Device per core: sim = fT_loc.T @ fT (float32r matmuls, PSUM fp32),
    E = exp((sim-1)/T) on ScalarE, then two fused multiply-reduce passes on
    VectorE: U_n = sum(E * delta), S_n = sum(sim * pm).
  - Host: loss_i = log(U_i) + (1 - S_i/P_i)/T, mean over all rows.

The row-max subtraction in the reference cancels analytically; the constant
shift 1/T (sim_ii ~ 1) keeps exp() in range. delta reduces from the NxN rank
count to V * c_small[i%B, j%B] where c_small[a,b] = #{k: mask[b,k] < mask[b,a]},
computed exactly on host with sort+searchsorted over the fp32 mask (same
comparison semantics as the reference).
"""
import math

import numpy as np

import concourse.bass as bass
import concourse.mybir as mybir
from concourse import tile
from concourse.tile import ScopedClock
from concourse.bass_utils import run_bass_kernel_spmd

TEMPERATURE = 0.07
KRNL_SIGMA = 1.0
B, V, D = 2048, 2, 512
N = B * V
NCORES = 8
R = N // NCORES          # 512 anchor rows per core
MT = R // 128            # 4 partition tiles of local rows
NT = N // 512            # 8 column tiles
KT = D // 128            # 4 contraction tiles
CW = 2560                # stored mask columns: diag patch 512 + slice 2048

_F32 = mybir.dt.float32
_F32R = mybir.dt.float32r


def _patch_tile_drain():
    """Split the Tile tail-drain's sem waits across sync nops (this walrus
    build rejects >2 sync waits on one CTRL instruction)."""
    if getattr(tile.TileContext, "_ant_drain_patched", False):
        return

    def _drain_and_barrier(self, tick_clock, wait_clock):
        nc = self.nc
        collector = nc.sync.nop(nofuse=True)
        wait_clock.add_sem_waits(
            collector.ins, ScopedClock({None: tick_clock.global_clock})
        )
        si = collector.ins.sync_info
        waits = list(si.on_wait) if si and si.on_wait else []
        if si and waits:
            si.on_wait = waits[:1]
        for w in waits[1:]:
            n = nc.sync.nop(nofuse=True)
            n.ins.sync_info = mybir.SyncInfo(on_wait=[w], on_update=[])
        nc.sync.drain()
        nc.all_engine_barrier()
        assert self.sems is not None
        popped = nc._tile_sem_poison_stack.pop()
        assert popped is self._sem_poison
        nc.clear_and_free_semaphores(list(self.sems.allocated().values()))
        nc.all_engine_barrier()

    tile.TileContext._drain_and_barrier = _drain_and_barrier
    tile.TileContext._ant_drain_patched = True


# ---------------------------------------------------------------- host prep

def _mask_small(labels: np.ndarray) -> np.ndarray:
    x = labels.reshape(-1, 1).astype(np.float32)
    d = x - x.T
    return (np.exp(-(d * d) / np.float32(2.0 * KRNL_SIGMA**2)) /
            np.float32(math.sqrt(2.0 * math.pi) * KRNL_SIGMA)).astype(np.float32)


def _c_small(msk: np.ndarray) -> np.ndarray:
    """c[a,b] = #{k : msk[b,k] < msk[b,a]} (strict, fp32 tie semantics)."""
    out = np.empty(msk.shape, dtype=np.float32)
    srt = np.sort(msk, axis=1)
    for b in range(msk.shape[0]):
        out[:, b] = np.searchsorted(srt[b], msk[b], side="left")
    return out


def host_prep(features: np.ndarray, labels: np.ndarray):
    feats = np.transpose(features, (1, 0, 2)).reshape(N, D).astype(np.float32)
    msk = _mask_small(labels)
    delta_small = (np.float32(V) * _c_small(msk)).astype(np.float32)

    in_maps = []
    P_all = np.empty((NCORES, R), dtype=np.float32)
    for c in range(NCORES):
        rows = np.arange(c * R, (c + 1) * R)
        a_idx = rows % B
        jglob = (np.arange(N) + c * R) % N          # rotated column order
        jb = jglob % B

        fT = np.ascontiguousarray(feats[jglob].T)   # [D, N] fp32

        d_loc = delta_small[np.ix_(a_idx, jb)]      # [R, N]
        p_loc = msk[np.ix_(a_idx, jb)]
        rr = np.arange(R)
        d_loc[rr, rr] = 0.0                         # diagonal is rotated col j'=r
        p_loc[rr, rr] = 0.0
        P_all[c] = p_loc.sum(axis=1, dtype=np.float32)

        in_maps.append({
            "fT": fT,
            "dcomb": np.ascontiguousarray(d_loc[:, :CW]),
            "pcomb": np.ascontiguousarray(p_loc[:, :CW]),
        })
    return in_maps, P_all


# ------------------------------------------------------------- device build

def _split_sync_waits(nc: bass.Bass, limit: int = 1):
    """This walrus build rejects instructions carrying more than `limit` sem
    waits; move the overflow onto preceding same-engine nops (engines run in
    program order, so waiting on an earlier nop is equivalent)."""
    import bass_rust
    uid = [0]
    for f in nc.m.functions:
        for bb in f.blocks:
            new_list = []
            for inst in bb.instructions:
                si = inst.sync_info
                waits = list(si.on_wait) if si and si.on_wait else []
                if len(waits) > limit:
                    for i in range(0, len(waits) - limit, limit):
                        chunk = waits[i:i + limit]
                        nop = bass_rust.InstNoOp(
                            name=f"I-waitsplit-{uid[0]}", engine=inst.engine)
                        uid[0] += 1
                        nop.sync_info = mybir.SyncInfo(
                            on_wait=chunk, on_update=[])
                        nc.register_instruction(nop)
                        new_list.append(nop)
                    si.on_wait = waits[len(waits) - limit:]
                new_list.append(inst)
            bb.instructions[:] = new_list


def build_nc() -> bass.Bass:
    _patch_tile_drain()
    nc = bass.Bass("TRN2", target_bir_lowering=False, debug=False,
                   num_devices=NCORES)
    fT_d = nc.dram_tensor("fT", [D, N], _F32R, kind="ExternalInput")
    d_d = nc.dram_tensor("dcomb", [R, CW], _F32, kind="ExternalInput")
    p_d = nc.dram_tensor("pcomb", [R, CW], _F32, kind="ExternalInput")
    u_d = nc.dram_tensor("Uparts", [MT, 128, NT], _F32, kind="ExternalOutput")
    s_d = nc.dram_tensor("Sparts", [MT, 128, NT], _F32, kind="ExternalOutput")

    inv_t = float(1.0 / TEMPERATURE)

    with tile.TileContext(nc) as tc:
        with (
            tc.tile_pool(name="feat", bufs=1) as feat_pool,
            tc.tile_pool(name="masks", bufs=1) as mask_pool,
            tc.tile_pool(name="work", bufs=3) as work_pool,
            tc.tile_pool(name="acc", bufs=2) as acc_pool,
            tc.tile_pool(name="psum", bufs=4, space="PSUM") as psum_pool,
        ):
            bias_t = feat_pool.tile([128, 1], _F32, name="bias")
            nc.vector.memset(bias_t[:], -inv_t)
            fT_t = []
            for k in range(KT):
                ft = feat_pool.tile([128, N], _F32R, name=f"ft{k}")
                nc.sync.dma_start(ft[:], fT_d[k * 128:(k + 1) * 128, :])
                fT_t.append(ft)
            d_t, p_t = [], []
            for m in range(MT):
                dt_ = mask_pool.tile([128, CW], _F32, name=f"dt{m}")
                nc.sync.dma_start(dt_[:], d_d[m * 128:(m + 1) * 128, :])
                d_t.append(dt_)
                pt_ = mask_pool.tile([128, CW], _F32, name=f"pt{m}")
                nc.sync.dma_start(pt_[:], p_d[m * 128:(m + 1) * 128, :])
                p_t.append(pt_)

            for m in range(MT):
                up = acc_pool.tile([128, NT], _F32, name=f"up{m}")
                sp = acc_pool.tile([128, NT], _F32, name=f"sp{m}")
                for n in range(NT):
                    ps = psum_pool.tile([128, 512], _F32, name="ps")
                    for k in range(KT):
                        nc.tensor.matmul(
                            ps[:],
                            lhsT=fT_t[k][:, m * 128:(m + 1) * 128],
                            rhs=fT_t[k][:, n * 512:(n + 1) * 512],
                            start=(k == 0),
                            stop=(k == KT - 1),
                        )
                    e = work_pool.tile([128, 512], _F32, name="e")
                    nc.scalar.activation(
                        e[:], ps[:], mybir.ActivationFunctionType.Exp,
                        bias=bias_t[:], scale=inv_t,
                    )
                    col0 = n * 512 if n <= 4 else (n - 4) * 512
                    scr = work_pool.tile([128, 512], _F32, name="scr")
                    nc.vector.scalar_tensor_tensor(
                        out=scr[:], in0=e[:], scalar=1.0,
                        in1=d_t[m][:, col0:col0 + 512],
                        op0=mybir.AluOpType.mult, op1=mybir.AluOpType.mult,
                        accum_out=up[:, n:n + 1],
                    )
                    scr2 = work_pool.tile([128, 512], _F32, name="scr2")
                    nc.vector.scalar_tensor_tensor(
                        out=scr2[:], in0=ps[:], scalar=1.0,
                        in1=p_t[m][:, col0:col0 + 512],
                        op0=mybir.AluOpType.mult, op1=mybir.AluOpType.mult,
                        accum_out=sp[:, n:n + 1],
                    )
                nc.sync.dma_start(u_d[m], up[:])
                nc.sync.dma_start(s_d[m], sp[:])
    _split_sync_waits(nc)
    return nc


# ------------------------------------------------------------------- kernel

def _postprocess(results, P_all: np.ndarray) -> np.ndarray:
    loss_rows = []
    for c in range(NCORES):
        U = results[c]["Uparts"].reshape(R, NT).sum(axis=1, dtype=np.float32)
        S = results[c]["Sparts"].reshape(R, NT).sum(axis=1, dtype=np.float32)
        P = P_all[c]
        loss_rows.append(np.log(U) + (np.float32(1.0) - S / P) / np.float32(TEMPERATURE))
    loss = np.concatenate(loss_rows).astype(np.float32)
    return np.float32(loss.mean())


def kernel(features: np.ndarray, labels: np.ndarray) -> np.ndarray:
    features = np.asarray(features, dtype=np.float32)
    labels = np.asarray(labels, dtype=np.float32)
    in_maps, P_all = host_prep(features, labels)
    nc = build_nc()
    res = run_bass_kernel_spmd(nc, in_maps, list(range(NCORES)))
    return np.asarray(_postprocess(res.results, P_all), dtype=np.float32)



# revision 46
# speedup vs baseline: 1412.3543x; 1412.3543x over previous
"""Trainium2 Bass kernel for KernelizedSupCon loss (B=2048, V=2, D=512, 8 cores).

Strategy (data-parallel over anchor rows, per sharding hint):
  - N = B*V = 4096 anchor rows; core c owns rows [c*512, (c+1)*512).
  - Host precomputes, per core: transposed/rotated features fT (fp8, laid out
    for DoubleRow matmuls), Lc = T*ln(delta) fp16 and the positive-mask pm
    fp8 as row-blocks (rotated so the diagonal block is always column-tile 0
    -> one uniform SPMD program), and host-side row sums / corrections.
  - Device per core, per [128,1024] psum tile: ps = fT_loc.T @ fT via 8
    fp8 DoubleRow matmuls (256-deep contraction each) plus ident.T @ Lc
    (2 fp16 matmuls) so ps = sim + T*ln(delta); ScalarE computes
    exp((ps - C0)/T) = exp((sim-C0)/T)*delta with a free accum_out giving
    the U row-sums; VectorE does one fused multiply-reduce for
    S' = sum(ps * pm) (host subtracts the exactly-known sum(pm*Lc)).
  - PE warmup matmuls run during the input DMA window so the HAM clock gate
    is released before the real matmuls start; inputs are shipped in wide-row
    contiguous layouts (the DMA emits one packet per descriptor row) in an
    order tuned so operands arrive just ahead of their consumers.
  - Host: loss_i = log(U_i) + (C0-1)/T + (1 - S_i/P_i)/T, mean over rows.

The row-max subtraction in the reference cancels analytically; the shift C0
keeps exp() and the fp16 scratch in range. delta = V * c_small[i%B, j%B]
where c_small[a,b] = #{k: mask[b,k] < mask[b,a]} is computed exactly on host
with sort+searchsorted over the fp32 mask (same comparison semantics as the
reference).
"""
import math

import numpy as np

import concourse.bass as bass
import concourse.mybir as mybir
from concourse import tile
from concourse.tile import ScopedClock
from concourse.bass_utils import run_bass_kernel_spmd

TEMPERATURE = 0.07
KRNL_SIGMA = 1.0
B, V, D = 2048, 2, 512
N = B * V
NCORES = 8
R = N // NCORES          # 512 anchor rows per core
MT = R // 128            # 4 partition tiles of local rows
NT = N // 512            # 8 column tiles (512 wide)
NP = NT // 2             # 4 psum pair-tiles (1024 wide)
KT = D // 128            # 4 contraction tiles
CW = 2560                # stored mask columns: diag patch 512 + slice 2048
C0 = 0.4                 # exp shift: E' = exp((sim - C0)/T); host re-adds (C0-1)/T
LC_CLAMP = -8.0          # T*ln(0) stand-in; exp((sim-8-C0)/T) == 0 in fp32
NWARM = 7                # PE warmup matmuls issued during the DMA window
SCOLS = 5                # S accumulator columns per m: pairs (0,1),(2,3),4,5,(6,7)
CHW = (1024, 1024, 512)  # mask chunk widths (stored cols 0:1024, 1024:2048, 2048:2560)

_F32 = mybir.dt.float32
_F16 = mybir.dt.float16


def _patch_tile_drain():
    """Split the Tile tail-drain's sem waits across sync nops (this walrus
    build rejects >2 sync waits on one CTRL instruction)."""
    if getattr(tile.TileContext, "_ant_drain_patched", False):
        return

    def _drain_and_barrier(self, tick_clock, wait_clock):
        nc = self.nc
        collector = nc.sync.nop(nofuse=True)
        wait_clock.add_sem_waits(
            collector.ins, ScopedClock({None: tick_clock.global_clock})
        )
        si = collector.ins.sync_info
        waits = list(si.on_wait) if si and si.on_wait else []
        if si and waits:
            si.on_wait = waits[:1]
        for w in waits[1:]:
            n = nc.sync.nop(nofuse=True)
            n.ins.sync_info = mybir.SyncInfo(on_wait=[w], on_update=[])
        nc.sync.drain()
        nc.all_engine_barrier()
        assert self.sems is not None
        popped = nc._tile_sem_poison_stack.pop()
        assert popped is self._sem_poison
        nc.clear_and_free_semaphores(list(self.sems.allocated().values()))
        nc.all_engine_barrier()

    tile.TileContext._drain_and_barrier = _drain_and_barrier
    tile.TileContext._ant_drain_patched = True


# ---------------------------------------------------------------- host prep

def _mask_small(labels: np.ndarray) -> np.ndarray:
    x = labels.reshape(-1, 1).astype(np.float32)
    d = x - x.T
    return (np.exp(-(d * d) / np.float32(2.0 * KRNL_SIGMA**2)) /
            np.float32(math.sqrt(2.0 * math.pi) * KRNL_SIGMA)).astype(np.float32)


def _c_small(msk: np.ndarray) -> np.ndarray:
    """c[a,b] = #{k : msk[b,k] < msk[b,a]} (strict, fp32 tie semantics)."""
    out = np.empty(msk.shape, dtype=np.float32)
    srt = np.sort(msk, axis=1)
    for b in range(msk.shape[0]):
        out[:, b] = np.searchsorted(srt[b], msk[b], side="left")
    return out


# column tile -> start column in the CW-wide stored layout
def _col0(n: int) -> int:
    return n * 512 if n <= 4 else (n - 4) * 512


def host_prep(features: np.ndarray, labels: np.ndarray):
    feats = np.transpose(features, (1, 0, 2)).reshape(N, D).astype(np.float32)
    msk = _mask_small(labels)
    delta_small = np.float32(V) * _c_small(msk)
    with np.errstate(divide="ignore"):
        lc_small = np.where(
            delta_small > 0.0,
            np.float32(TEMPERATURE) * np.log(delta_small, dtype=np.float32),
            np.float32(LC_CLAMP),
        ).astype(np.float32)

    in_maps = []
    P_all = np.empty((NCORES, R), dtype=np.float64)
    corr_all = np.empty((NCORES, R), dtype=np.float64)
    ident = np.eye(128, dtype=np.float16)
    for c in range(NCORES):
        rows = np.arange(c * R, (c + 1) * R)
        a_idx = rows % B
        jglob = (np.arange(N) + c * R) % N          # rotated column order
        jb = jglob % B

        fT = np.ascontiguousarray(feats[jglob].T.astype(np.float16))  # [D, N]

        lc_loc = lc_small[np.ix_(a_idx, jb)]        # [R, N]
        p_loc = msk[np.ix_(a_idx, jb)]
        rr = np.arange(R)
        lc_loc[rr, rr] = LC_CLAMP                   # diagonal is rotated col j'=r
        p_loc[rr, rr] = 0.0

        lc16 = lc_loc[:, :CW].astype(np.float16)

        # expand the stored layout back to the device's [R, N] view (exact
        # shipped values) for the host-side corrections
        cols = np.concatenate(
            [np.arange(_col0(n), _col0(n) + 512) for n in range(NT)])
        lc_dev = lc16[:, cols].astype(np.float64)

        # chunk-major, fully contiguous layouts with wide rows (the DMA emits
        # one packet per descriptor row, so wide rows = high bandwidth):
        #   fT fp8 per 1024-column chunk as [128, 4, 1024]: the four 128-row
        #   d-slices side by side, ordered for DoubleRow k-pairs (row = 4KB);
        #   Lc fp16 [128, 2560] per m (row = 5KB); pm fp8 [128, 2560] per m.
        f8 = fT.astype(mybir.dt.np(mybir.dt.float8e4))
        ftch = np.empty((4, 128, KT, 1024), dtype=f8.dtype)
        for ch in range(4):
            for k in range(KT):
                ftch[ch, :, k] = f8[k * 128:(k + 1) * 128,
                                    ch * 1024:(ch + 1) * 1024]
        pm8 = p_loc[:, :CW].astype(f8.dtype)
        lcm = np.empty((MT, 128, CW), dtype=np.float16)
        pmm = np.empty((MT, 128, CW), dtype=f8.dtype)
        for m in range(MT):
            sl = slice(m * 128, (m + 1) * 128)
            lcm[m] = lc16[sl]
            pmm[m] = pm8[sl]
        # host-side corrections use the exact shipped values
        pm_dev8 = pm8[:, cols].astype(np.float64)
        P_all[c] = pm_dev8.sum(axis=1)
        corr_all[c] = (pm_dev8 * lc_dev).sum(axis=1)
        in_maps.append({
            "fT": ftch,
            "ident": ident,
            "lcm": lcm,
            "pmm": pmm,
        })
    return in_maps, (P_all, corr_all)


# ------------------------------------------------------------- device build

def _split_sync_waits(nc: bass.Bass, limit: int = 1):
    """This walrus build rejects instructions carrying more than `limit` sem
    waits; move the overflow onto preceding same-engine nops (engines run in
    program order, so waiting on an earlier nop is equivalent)."""
    import bass_rust
    uid = [0]
    for f in nc.m.functions:
        for bb in f.blocks:
            new_list = []
            for inst in bb.instructions:
                si = inst.sync_info
                waits = list(si.on_wait) if si and si.on_wait else []
                if len(waits) > limit:
                    for i in range(0, len(waits) - limit, limit):
                        chunk = waits[i:i + limit]
                        nop = bass_rust.InstNoOp(
                            name=f"I-waitsplit-{uid[0]}", engine=inst.engine)
                        uid[0] += 1
                        nop.sync_info = mybir.SyncInfo(
                            on_wait=chunk, on_update=[])
                        nc.register_instruction(nop)
                        new_list.append(nop)
                    si.on_wait = waits[len(waits) - limit:]
                new_list.append(inst)
            bb.instructions[:] = new_list


def build_nc() -> bass.Bass:
    _patch_tile_drain()
    nc = bass.Bass("TRN2", target_bir_lowering=False, debug=False,
                   num_devices=NCORES)
    _F8 = mybir.dt.float8e4
    fT_d = nc.dram_tensor("fT", [4, 128, KT, 1024], _F8, kind="ExternalInput")
    id_d = nc.dram_tensor("ident", [128, 128], _F16, kind="ExternalInput")
    lc_d = nc.dram_tensor("lcm", [MT, 128, CW], _F16, kind="ExternalInput")
    pm_d = nc.dram_tensor("pmm", [MT, 128, CW], _F8, kind="ExternalInput")
    u_d = nc.dram_tensor("Uparts", [MT, 128, NP], _F32, kind="ExternalOutput")
    s_d = nc.dram_tensor("Sparts", [MT, 128, SCOLS], _F32, kind="ExternalOutput")

    inv_t = float(1.0 / TEMPERATURE)



    with tile.TileContext(nc) as tc:
        with (
            tc.tile_pool(name="feat", bufs=1) as feat_pool,
            tc.tile_pool(name="masks", bufs=1) as mask_pool,
            tc.tile_pool(name="ework", bufs=3) as e_pool,
            tc.tile_pool(name="swork", bufs=3) as s_pool,
            tc.tile_pool(name="acc", bufs=1) as acc_pool,
            tc.tile_pool(name="psum", bufs=3, space="PSUM") as psum_pool,
            tc.tile_pool(name="pswarm", bufs=1, space="PSUM") as warm_pool,
        ):
            bias_t = feat_pool.tile([128, 1], _F32, name="bias")
            nc.vector.memset(bias_t[:], -C0 * inv_t)
            id_t = feat_pool.tile([128, 128], _F16, name="ident")
            nc.sync.dma_start(id_t[:], id_d[:])
            warm_t = feat_pool.tile([128, 512], _F16, name="warm")
            nc.vector.memset(warm_t[:], 0.001)

            # PE warmup during the input-DMA window: releases the HAM clock
            # gate before the real matmuls arrive.
            psw = warm_pool.tile([128, 512], _F32, name="psw")
            for i in range(NWARM):
                nc.tensor.matmul(psw[:], lhsT=warm_t[:, :128], rhs=warm_t[:],
                                 start=(i == 0), stop=(i == NWARM - 1))

            # wide-row input tiles: one [128, 4(k-slices), 1024] fp8 tile per
            # fT column chunk, Lc fp16 / pm fp8 [128, CW] tiles per m.
            ftc = [feat_pool.tile([128, KT, 1024], _F8, name=f"ftc{ch}")
                   for ch in range(4)]
            # masks split into A (stored cols 0:1024, needed by pair 0) and
            # B (1024:2560) tiles so pair 0 starts after only the A halves
            lcA = [mask_pool.tile([128, 1024], _F16, name=f"lcA{m}")
                   for m in range(MT)]
            lcB = [mask_pool.tile([128, 1536], _F16, name=f"lcB{m}")
                   for m in range(MT)]
            pmA = [mask_pool.tile([128, 1024], _F8, name=f"pmA{m}")
                   for m in range(MT)]
            pmB = [mask_pool.tile([128, 1536], _F8, name=f"pmB{m}")
                   for m in range(MT)]

            def lc_ap(m, n):
                off = _col0(n)
                return (lcA[m][:, off:off + 512] if off < 1024
                        else lcB[m][:, off - 1024:off - 1024 + 512])

            def pm_ap(m, n, width=512):
                off = _col0(n)
                return (pmA[m][:, off:off + width] if off < 1024
                        else pmB[m][:, off - 1024:off - 1024 + width])

            # DMA issue order tuned so the PE rarely waits
            nc.sync.dma_start(ftc[0][:], fT_d[0])
            for m in range(MT):
                nc.sync.dma_start(lcA[m][:], lc_d[m, :, 0:1024])
                nc.sync.dma_start(pmA[m][:], pm_d[m, :, 0:1024])
            nc.sync.dma_start(ftc[1][:], fT_d[1])
            for m in range(MT):
                nc.sync.dma_start(lcB[m][:], lc_d[m, :, 1024:CW])
                nc.sync.dma_start(pmB[m][:], pm_d[m, :, 1024:CW])
            nc.sync.dma_start(ftc[2][:], fT_d[2])
            nc.sync.dma_start(ftc[3][:], fT_d[3])

            up = [acc_pool.tile([128, NP], _F32, name=f"up{m}") for m in range(MT)]
            sp = [acc_pool.tile([128, SCOLS], _F32, name=f"sp{m}")
                  for m in range(MT)]

            for pr in range(NP):
                n0, n1 = 2 * pr, 2 * pr + 1
                for m in range(MT):
                    ps = psum_pool.tile([128, 1024], _F32, name="ps")
                    # all DoubleRow matmuls first, then both fp16 Lc-adds:
                    # fewer PE weight-path mode switches
                    for half, n in ((0, n0), (1, n1)):
                        sl = ps[:, half * 512:(half + 1) * 512]
                        for kp in range(KT // 2):
                            nc.tensor.matmul(
                                sl,
                                lhsT=ftc[0][:, 2 * kp:2 * kp + 2,
                                            m * 128:(m + 1) * 128],
                                rhs=ftc[n // 2][:, 2 * kp:2 * kp + 2,
                                                (n % 2) * 512:(n % 2) * 512 + 512],
                                start=(kp == 0),
                                stop=False,
                                perf_mode=mybir.MatmulPerfMode.DoubleRow,
                            )
                    for half, n in ((0, n0), (1, n1)):
                        nc.tensor.matmul(
                            ps[:, half * 512:(half + 1) * 512],
                            lhsT=id_t[:],
                            rhs=lc_ap(m, n),
                            start=False,
                            stop=True,
                        )
                    # ScalarE: exp((ps - C0)/T) = E' * delta; accum -> U
                    e = e_pool.tile([128, 1024], _F16, name="e")
                    nc.scalar.activation(
                        e[:], ps[:], mybir.ActivationFunctionType.Exp,
                        bias=bias_t[:], scale=inv_t,
                        accum_out=up[m][:, pr:pr + 1],
                    )
                    # VectorE: S' = sum(pm * ps); contiguous pairs in one op
                    if pr == 2:  # stored cols for n=4,5 are not adjacent
                        for half, n, sc in ((0, 4, 2), (1, 5, 3)):
                            scr = s_pool.tile([128, 512], _F16, name="scr")
                            nc.vector.scalar_tensor_tensor(
                                out=scr[:], in0=ps[:, half * 512:(half + 1) * 512],
                                scalar=1.0,
                                in1=pm_ap(m, n),
                                op0=mybir.AluOpType.mult,
                                op1=mybir.AluOpType.mult,
                                accum_out=sp[m][:, sc:sc + 1],
                            )
                    else:
                        sc = {0: 0, 1: 1, 3: 4}[pr]
                        scr = s_pool.tile([128, 1024], _F16, name="scrw")
                        nc.vector.scalar_tensor_tensor(
                            out=scr[:], in0=ps[:], scalar=1.0,
                            in1=pm_ap(m, n0, width=1024),
                            op0=mybir.AluOpType.mult,
                            op1=mybir.AluOpType.mult,
                            accum_out=sp[m][:, sc:sc + 1],
                        )
                    if pr == NP - 1:
                        nc.sync.dma_start(u_d[m], up[m][:])
                        nc.sync.dma_start(s_d[m], sp[m][:])
    _split_sync_waits(nc)
    return nc


# ------------------------------------------------------------------- kernel

def _postprocess(results, host_aux) -> np.ndarray:
    P_all, corr_all = host_aux
    log_shift = (C0 - 1.0) / TEMPERATURE
    loss_rows = []
    for c in range(NCORES):
        U = results[c]["Uparts"].reshape(R, NP).sum(axis=1, dtype=np.float64)
        Sp = results[c]["Sparts"].reshape(R, SCOLS).sum(axis=1, dtype=np.float64)
        S = Sp - corr_all[c]
        P = P_all[c]
        loss_rows.append(np.log(U) + log_shift + (1.0 - S / P) / TEMPERATURE)
    loss = np.concatenate(loss_rows)
    return np.float32(loss.mean())


def kernel(features: np.ndarray, labels: np.ndarray) -> np.ndarray:
    features = np.asarray(features, dtype=np.float32)
    labels = np.asarray(labels, dtype=np.float32)
    in_maps, host_aux = host_prep(features, labels)
    nc = build_nc()
    res = run_bass_kernel_spmd(nc, in_maps, list(range(NCORES)))
    return np.asarray(_postprocess(res.results, host_aux), dtype=np.float32)


# revision 49
# speedup vs baseline: 1435.8877x; 1.0167x over previous
"""Trainium2 Bass kernel for KernelizedSupCon loss (B=2048, V=2, D=512, 8 cores).

Strategy (data-parallel over anchor rows, per sharding hint):
  - N = B*V = 4096 anchor rows; core c owns rows [c*512, (c+1)*512).
  - Host precomputes, per core: transposed/rotated features fT (fp8, laid out
    for DoubleRow matmuls), Lc = T*ln(delta) fp16 and the positive-mask pm
    fp8 as row-blocks (rotated so the diagonal block is always column-tile 0
    -> one uniform SPMD program), and host-side row sums / corrections.
  - Device per core, per [128,1024] psum tile: ps = fT_loc.T @ fT via 8
    fp8 DoubleRow matmuls (256-deep contraction each) plus ident.T @ Lc
    (2 fp16 matmuls) so ps = sim + T*ln(delta); ScalarE computes
    exp((ps - C0)/T) = exp((sim-C0)/T)*delta with a free accum_out giving
    the U row-sums; VectorE does one fused multiply-reduce for
    S' = sum(ps * pm) (host subtracts the exactly-known sum(pm*Lc)).
  - PE warmup matmuls run during the input DMA window so the HAM clock gate
    is released before the real matmuls start; inputs are shipped in wide-row
    contiguous layouts (the DMA emits one packet per descriptor row) in an
    order tuned so operands arrive just ahead of their consumers.
  - Host: loss_i = log(U_i) + (C0-1)/T + (1 - S_i/P_i)/T, mean over rows.

The row-max subtraction in the reference cancels analytically; the shift C0
keeps exp() and the fp16 scratch in range. delta = V * c_small[i%B, j%B]
where c_small[a,b] = #{k: mask[b,k] < mask[b,a]} is computed exactly on host
with sort+searchsorted over the fp32 mask (same comparison semantics as the
reference).
"""
import math

import numpy as np

import concourse.bass as bass
import concourse.mybir as mybir
from concourse import tile
from concourse.tile import ScopedClock
from concourse.bass_utils import run_bass_kernel_spmd

TEMPERATURE = 0.07
KRNL_SIGMA = 1.0
B, V, D = 2048, 2, 512
N = B * V
NCORES = 8
R = N // NCORES          # 512 anchor rows per core
MT = R // 128            # 4 partition tiles of local rows
NT = N // 512            # 8 column tiles (512 wide)
NP = NT // 2             # 4 psum pair-tiles (1024 wide)
KT = D // 128            # 4 contraction tiles
CW = 2560                # stored mask columns: diag patch 512 + slice 2048
C0 = 0.4                 # exp shift: E' = exp((sim - C0)/T); host re-adds (C0-1)/T
LC_CLAMP = -8.0          # T*ln(0) stand-in; exp((sim-8-C0)/T) == 0 in fp32
NWARM = 7                # PE warmup matmuls issued during the DMA window
SCOLS = 5                # S accumulator columns per m: pairs (0,1),(2,3),4,5,(6,7)
CHW = (1024, 1024, 512)  # mask chunk widths (stored cols 0:1024, 1024:2048, 2048:2560)

_F32 = mybir.dt.float32
_F16 = mybir.dt.float16


def _patch_tile_drain():
    """Split the Tile tail-drain's sem waits across sync nops (this walrus
    build rejects >2 sync waits on one CTRL instruction)."""
    if getattr(tile.TileContext, "_ant_drain_patched", False):
        return

    def _drain_and_barrier(self, tick_clock, wait_clock):
        nc = self.nc
        collector = nc.sync.nop(nofuse=True)
        wait_clock.add_sem_waits(
            collector.ins, ScopedClock({None: tick_clock.global_clock})
        )
        si = collector.ins.sync_info
        waits = list(si.on_wait) if si and si.on_wait else []
        if si and waits:
            si.on_wait = waits[:1]
        for w in waits[1:]:
            n = nc.sync.nop(nofuse=True)
            n.ins.sync_info = mybir.SyncInfo(on_wait=[w], on_update=[])
        nc.sync.drain()
        nc.all_engine_barrier()
        assert self.sems is not None
        popped = nc._tile_sem_poison_stack.pop()
        assert popped is self._sem_poison
        nc.clear_and_free_semaphores(list(self.sems.allocated().values()))
        nc.all_engine_barrier()

    tile.TileContext._drain_and_barrier = _drain_and_barrier
    tile.TileContext._ant_drain_patched = True


# ---------------------------------------------------------------- host prep

def _mask_small(labels: np.ndarray) -> np.ndarray:
    x = labels.reshape(-1, 1).astype(np.float32)
    d = x - x.T
    return (np.exp(-(d * d) / np.float32(2.0 * KRNL_SIGMA**2)) /
            np.float32(math.sqrt(2.0 * math.pi) * KRNL_SIGMA)).astype(np.float32)


def _c_small(msk: np.ndarray) -> np.ndarray:
    """c[a,b] = #{k : msk[b,k] < msk[b,a]} (strict, fp32 tie semantics)."""
    out = np.empty(msk.shape, dtype=np.float32)
    srt = np.sort(msk, axis=1)
    for b in range(msk.shape[0]):
        out[:, b] = np.searchsorted(srt[b], msk[b], side="left")
    return out


# column tile -> start column in the CW-wide stored layout
def _col0(n: int) -> int:
    return n * 512 if n <= 4 else (n - 4) * 512


def host_prep(features: np.ndarray, labels: np.ndarray):
    feats = np.transpose(features, (1, 0, 2)).reshape(N, D).astype(np.float32)
    msk = _mask_small(labels)
    delta_small = np.float32(V) * _c_small(msk)
    with np.errstate(divide="ignore"):
        lc_small = np.where(
            delta_small > 0.0,
            np.float32(TEMPERATURE) * np.log(delta_small, dtype=np.float32),
            np.float32(LC_CLAMP),
        ).astype(np.float32)

    in_maps = []
    P_all = np.empty((NCORES, R), dtype=np.float64)
    corr_all = np.empty((NCORES, R), dtype=np.float64)
    ident = np.eye(128, dtype=np.float16)
    for c in range(NCORES):
        rows = np.arange(c * R, (c + 1) * R)
        a_idx = rows % B
        jglob = (np.arange(N) + c * R) % N          # rotated column order
        jb = jglob % B

        fT = np.ascontiguousarray(feats[jglob].T.astype(np.float16))  # [D, N]

        lc_loc = lc_small[np.ix_(a_idx, jb)]        # [R, N]
        p_loc = msk[np.ix_(a_idx, jb)]
        rr = np.arange(R)
        lc_loc[rr, rr] = LC_CLAMP                   # diagonal is rotated col j'=r
        p_loc[rr, rr] = 0.0

        lc16 = lc_loc[:, :CW].astype(np.float16)

        # expand the stored layout back to the device's [R, N] view (exact
        # shipped values) for the host-side corrections
        cols = np.concatenate(
            [np.arange(_col0(n), _col0(n) + 512) for n in range(NT)])
        lc_dev = lc16[:, cols].astype(np.float64)

        # chunk-major, fully contiguous layouts with wide rows (the DMA emits
        # one packet per descriptor row, so wide rows = high bandwidth):
        #   fT fp8 per 1024-column chunk as [128, 4, 1024]: the four 128-row
        #   d-slices side by side, ordered for DoubleRow k-pairs (row = 4KB);
        #   Lc fp16 [128, 2560] per m (row = 5KB); pm fp8 [128, 2560] per m.
        f8 = fT.astype(mybir.dt.np(mybir.dt.float8e4))
        ftch = np.empty((4, 128, KT, 1024), dtype=f8.dtype)
        for ch in range(4):
            for k in range(KT):
                ftch[ch, :, k] = f8[k * 128:(k + 1) * 128,
                                    ch * 1024:(ch + 1) * 1024]
        pm8 = p_loc[:, :CW].astype(f8.dtype)
        lcm = np.empty((MT, 128, CW), dtype=np.float16)
        pmm = np.empty((MT, 128, CW), dtype=f8.dtype)
        for m in range(MT):
            sl = slice(m * 128, (m + 1) * 128)
            lcm[m] = lc16[sl]
            pmm[m] = pm8[sl]
        # host-side corrections use the exact shipped values
        pm_dev8 = pm8[:, cols].astype(np.float64)
        P_all[c] = pm_dev8.sum(axis=1)
        corr_all[c] = (pm_dev8 * lc_dev).sum(axis=1)
        in_maps.append({
            "fT": ftch,
            "ident": ident,
            "lcm": lcm,
            "pmm": pmm,
        })
    return in_maps, (P_all, corr_all)


# ------------------------------------------------------------- device build

def _split_sync_waits(nc: bass.Bass, limit: int = 1):
    """This walrus build rejects instructions carrying more than `limit` sem
    waits; move the overflow onto preceding same-engine nops (engines run in
    program order, so waiting on an earlier nop is equivalent)."""
    import bass_rust
    uid = [0]
    for f in nc.m.functions:
        for bb in f.blocks:
            new_list = []
            for inst in bb.instructions:
                si = inst.sync_info
                waits = list(si.on_wait) if si and si.on_wait else []
                if len(waits) > limit:
                    for i in range(0, len(waits) - limit, limit):
                        chunk = waits[i:i + limit]
                        nop = bass_rust.InstNoOp(
                            name=f"I-waitsplit-{uid[0]}", engine=inst.engine)
                        uid[0] += 1
                        nop.sync_info = mybir.SyncInfo(
                            on_wait=chunk, on_update=[])
                        nc.register_instruction(nop)
                        new_list.append(nop)
                    si.on_wait = waits[len(waits) - limit:]
                new_list.append(inst)
            bb.instructions[:] = new_list


def build_nc() -> bass.Bass:
    _patch_tile_drain()
    nc = bass.Bass("TRN2", target_bir_lowering=False, debug=False,
                   num_devices=NCORES)
    _F8 = mybir.dt.float8e4
    fT_d = nc.dram_tensor("fT", [4, 128, KT, 1024], _F8, kind="ExternalInput")
    id_d = nc.dram_tensor("ident", [128, 128], _F16, kind="ExternalInput")
    lc_d = nc.dram_tensor("lcm", [MT, 128, CW], _F16, kind="ExternalInput")
    pm_d = nc.dram_tensor("pmm", [MT, 128, CW], _F8, kind="ExternalInput")
    u_d = nc.dram_tensor("Uparts", [MT, 128, NP], _F32, kind="ExternalOutput")
    s_d = nc.dram_tensor("Sparts", [MT, 128, SCOLS], _F32, kind="ExternalOutput")

    inv_t = float(1.0 / TEMPERATURE)



    with tile.TileContext(nc) as tc:
        with (
            tc.tile_pool(name="feat", bufs=1) as feat_pool,
            tc.tile_pool(name="masks", bufs=1) as mask_pool,
            tc.tile_pool(name="ework", bufs=3) as e_pool,
            tc.tile_pool(name="swork", bufs=3) as s_pool,
            tc.tile_pool(name="acc", bufs=1) as acc_pool,
            tc.tile_pool(name="psum", bufs=3, space="PSUM") as psum_pool,
            tc.tile_pool(name="pswarm", bufs=1, space="PSUM") as warm_pool,
        ):
            bias_t = feat_pool.tile([128, 1], _F32, name="bias")
            nc.vector.memset(bias_t[:], -C0 * inv_t)
            id_t = feat_pool.tile([128, 128], _F16, name="ident")
            nc.sync.dma_start(id_t[:], id_d[:])
            warm_t = feat_pool.tile([128, 512], _F16, name="warm")
            nc.vector.memset(warm_t[:], 0.001)

            # PE warmup during the input-DMA window: releases the HAM clock
            # gate before the real matmuls arrive.
            psw = warm_pool.tile([128, 512], _F32, name="psw")
            for i in range(NWARM):
                nc.tensor.matmul(psw[:], lhsT=warm_t[:, :128], rhs=warm_t[:],
                                 start=(i == 0), stop=(i == NWARM - 1))

            # wide-row input tiles: one [128, 4(k-slices), 1024] fp8 tile per
            # fT column chunk, Lc fp16 / pm fp8 [128, CW] tiles per m.
            ftc = [feat_pool.tile([128, KT, 1024], _F8, name=f"ftc{ch}")
                   for ch in range(4)]
            # masks split into A (stored cols 0:1024, needed by pair 0) and
            # B (1024:2560) tiles so pair 0 starts after only the A halves
            lcA = [mask_pool.tile([128, 1024], _F16, name=f"lcA{m}")
                   for m in range(MT)]
            lcB = [mask_pool.tile([128, 1536], _F16, name=f"lcB{m}")
                   for m in range(MT)]
            pmA = [mask_pool.tile([128, 1024], _F8, name=f"pmA{m}")
                   for m in range(MT)]
            pmB = [mask_pool.tile([128, 1536], _F8, name=f"pmB{m}")
                   for m in range(MT)]

            def lc_ap(m, n):
                off = _col0(n)
                return (lcA[m][:, off:off + 512] if off < 1024
                        else lcB[m][:, off - 1024:off - 1024 + 512])

            def pm_ap(m, n, width=512):
                off = _col0(n)
                return (pmA[m][:, off:off + width] if off < 1024
                        else pmB[m][:, off - 1024:off - 1024 + width])

            # DMA issue order tuned so the PE rarely waits
            nc.sync.dma_start(lcA[0][:], lc_d[0, :, 0:1024])
            nc.sync.dma_start(ftc[0][:], fT_d[0])
            nc.sync.dma_start(pmA[0][:], pm_d[0, :, 0:1024])
            for m in range(1, MT):
                nc.sync.dma_start(lcA[m][:], lc_d[m, :, 0:1024])
                nc.sync.dma_start(pmA[m][:], pm_d[m, :, 0:1024])
            nc.sync.dma_start(ftc[1][:], fT_d[1])
            for m in range(MT):
                nc.sync.dma_start(lcB[m][:], lc_d[m, :, 1024:CW])
                nc.sync.dma_start(pmB[m][:], pm_d[m, :, 1024:CW])
            nc.sync.dma_start(ftc[2][:], fT_d[2])
            nc.sync.dma_start(ftc[3][:], fT_d[3])

            up = [acc_pool.tile([128, NP], _F32, name=f"up{m}") for m in range(MT)]
            sp = [acc_pool.tile([128, SCOLS], _F32, name=f"sp{m}")
                  for m in range(MT)]

            for pr in range(NP):
                n0, n1 = 2 * pr, 2 * pr + 1
                for m in range(MT):
                    ps = psum_pool.tile([128, 1024], _F32, name="ps")
                    # all DoubleRow matmuls first, then both fp16 Lc-adds:
                    # fewer PE weight-path mode switches
                    for half, n in ((0, n0), (1, n1)):
                        sl = ps[:, half * 512:(half + 1) * 512]
                        for kp in range(KT // 2):
                            nc.tensor.matmul(
                                sl,
                                lhsT=ftc[0][:, 2 * kp:2 * kp + 2,
                                            m * 128:(m + 1) * 128],
                                rhs=ftc[n // 2][:, 2 * kp:2 * kp + 2,
                                                (n % 2) * 512:(n % 2) * 512 + 512],
                                start=(kp == 0),
                                stop=False,
                                perf_mode=mybir.MatmulPerfMode.DoubleRow,
                            )
                    for half, n in ((0, n0), (1, n1)):
                        nc.tensor.matmul(
                            ps[:, half * 512:(half + 1) * 512],
                            lhsT=id_t[:],
                            rhs=lc_ap(m, n),
                            start=False,
                            stop=True,
                        )
                    # ScalarE: exp((ps - C0)/T) = E' * delta; accum -> U
                    e = e_pool.tile([128, 1024], _F16, name="e")
                    nc.scalar.activation(
                        e[:], ps[:], mybir.ActivationFunctionType.Exp,
                        bias=bias_t[:], scale=inv_t,
                        accum_out=up[m][:, pr:pr + 1],
                    )
                    # VectorE: S' = sum(pm * ps); contiguous pairs in one op
                    if pr == 2:  # stored cols for n=4,5 are not adjacent
                        for half, n, sc in ((0, 4, 2), (1, 5, 3)):
                            scr = s_pool.tile([128, 512], _F16, name="scr")
                            nc.vector.scalar_tensor_tensor(
                                out=scr[:], in0=ps[:, half * 512:(half + 1) * 512],
                                scalar=1.0,
                                in1=pm_ap(m, n),
                                op0=mybir.AluOpType.mult,
                                op1=mybir.AluOpType.mult,
                                accum_out=sp[m][:, sc:sc + 1],
                            )
                    else:
                        sc = {0: 0, 1: 1, 3: 4}[pr]
                        scr = s_pool.tile([128, 1024], _F16, name="scrw")
                        nc.vector.scalar_tensor_tensor(
                            out=scr[:], in0=ps[:], scalar=1.0,
                            in1=pm_ap(m, n0, width=1024),
                            op0=mybir.AluOpType.mult,
                            op1=mybir.AluOpType.mult,
                            accum_out=sp[m][:, sc:sc + 1],
                        )
                    if pr == NP - 1:
                        nc.sync.dma_start(u_d[m], up[m][:])
                        nc.sync.dma_start(s_d[m], sp[m][:])
    _split_sync_waits(nc)
    return nc


# ------------------------------------------------------------------- kernel

def _postprocess(results, host_aux) -> np.ndarray:
    P_all, corr_all = host_aux
    log_shift = (C0 - 1.0) / TEMPERATURE
    loss_rows = []
    for c in range(NCORES):
        U = results[c]["Uparts"].reshape(R, NP).sum(axis=1, dtype=np.float64)
        Sp = results[c]["Sparts"].reshape(R, SCOLS).sum(axis=1, dtype=np.float64)
        S = Sp - corr_all[c]
        P = P_all[c]
        loss_rows.append(np.log(U) + log_shift + (1.0 - S / P) / TEMPERATURE)
    loss = np.concatenate(loss_rows)
    return np.float32(loss.mean())


def kernel(features: np.ndarray, labels: np.ndarray) -> np.ndarray:
    features = np.asarray(features, dtype=np.float32)
    labels = np.asarray(labels, dtype=np.float32)
    in_maps, host_aux = host_prep(features, labels)
    nc = build_nc()
    res = run_bass_kernel_spmd(nc, in_maps, list(range(NCORES)))
    return np.asarray(_postprocess(res.results, host_aux), dtype=np.float32)
